# revision 15
# baseline (speedup 1.0000x reference)
"""Trainium2 Bass kernel for per-pixel temporal mode (background) extraction.

Problem: input [B=200, C=3, H=240, W=320] float32 with integer values in
[0, 256).  For each pixel, compute the mode over the batch dim (256-bin
histogram argmax, first-max-wins), broadcast it as `bg`, and fg = |x - bg|.

Strategy (8 NeuronCores, SPMD): shard the pixel axis P = C*H*W = 230400 into
8 slices of 28800.  Per core:

  Phase A (mode): transpose the [200, 28800] slice to pixel-major [128, 256]
  bf16 tiles (batch along the free dim, padded with 56 distinct sentinel
  values 256..311), bitonic-sort each row (36 all-ascending substages using
  mirror-first networks), then extract the longest equal-run with a
  multiply-add prefix scan and the packed score runpos*256 - value whose
  row-max encodes (count, smallest-value) with exactly the reference
  tie-breaking.

  Phase B (outputs): broadcast mode rows across partitions with a k=1 PE
  matmul, bg = broadcast, fg = |x - bg|, stream out.

kernel(input) -> (bg, fg), both [200, 3, 240, 320] float32.
"""

import os
import sys
from contextlib import ExitStack

import numpy as np

sys.path.insert(0, "/opt/trn_rl_repo")  # harmless if concourse already importable

import ml_dtypes

import concourse.bass as bass
import concourse.tile as tile
from concourse import bacc, mybir
from concourse.bass_utils import run_bass_kernel_spmd

AF = mybir.AluOpType
DT = mybir.dt

B = 200          # batch (frames)
BPAD = 208       # batch rows padded to mult of 16 for dma transpose
SORT_N = 256     # sort width (200 data + 56 sentinels)
N_CORES = 8
P_FULL = 3 * 240 * 320   # 230400
P_CORE = P_FULL // N_CORES  # 28800 pixels per core


def build_program(p_core=P_CORE, g=5, pixchunk_segs=15):
    """Build the SPMD Bass program for one core's [B, p_core] slice."""
    nseg = p_core // 128          # sort segments (128 pixels each)
    assert nseg % g == 0, (nseg, g)
    ngroups = nseg // g
    segs_per_chunk = pixchunk_segs
    assert nseg % segs_per_chunk == 0
    nchunk = nseg // segs_per_chunk
    chunk_pix = segs_per_chunk * 128

    nc = bacc.Bacc(
        "TRN2", target_bir_lowering=False, debug=False, num_devices=N_CORES
    )
    x = nc.declare_dram_parameter("x", [B, p_core], DT.float32, isOutput=False)
    bg = nc.declare_dram_parameter("bg", [B, p_core], DT.float32, isOutput=True)
    fg = nc.declare_dram_parameter("fg", [B, p_core], DT.float32, isOutput=True)

    # mode in pixel order, staging for the phase-B broadcast loads
    mode_d = nc.dram_tensor("mode_d", [p_core], DT.float32)

    # constants baked into the NEFF
    sent_np = np.broadcast_to(
        (256 + np.arange(56)).astype(ml_dtypes.bfloat16), (128, 56)
    ).copy()
    sent_c = nc.inline_tensor(sent_np, "sent_c")
    ident_c = nc.inline_tensor(np.eye(128, dtype=ml_dtypes.bfloat16), "ident_c")

    with tile.TileContext(nc) as tc, ExitStack() as ctx:
        consts = ctx.enter_context(tc.tile_pool(name="consts", bufs=1))
        sent_sb = consts.tile([128, 56], DT.bfloat16)
        nc.sync.dma_start(out=sent_sb[:], in_=sent_c[:])
        ident_sb = consts.tile([128, 128], DT.bfloat16)
        nc.sync.dma_start(out=ident_sb[:], in_=ident_c[:])

        # score_cols[p, s] = max packed score for pixel s*128+p
        score_cols = consts.tile([128, nseg], DT.float32)

        # ---- Phase A ----
        sortA = ctx.enter_context(tc.tile_pool(name="sortA", bufs=2))
        sortB = ctx.enter_context(tc.tile_pool(name="sortB", bufs=2))
        rlp = ctx.enter_context(tc.tile_pool(name="rl", bufs=2))
        xap = ctx.enter_context(tc.tile_pool(name="xa", bufs=3))
        psT = ctx.enter_context(tc.tile_pool(name="psT", bufs=4, space="PSUM"))

        gpix = g * 128
        row_splits = [(0, 128), (128, B)]
        for grp in range(ngroups):
            tA = sortA.tile([128, g, SORT_N], DT.bfloat16, tag="tA")
            tB = sortB.tile([128, g, SORT_N], DT.bfloat16, tag="tB")
            # load [B, gpix] fp32 in two row chunks, cast to bf16, PE-transpose
            # each [nr, 128] block into tA[:, k, r0:r0+nr]
            for (r0, r1) in row_splits:
                nr = r1 - r0
                xt = xap.tile([128, gpix], DT.float32, tag="xt")
                nc.sync.dma_start(
                    out=xt[0:nr, :], in_=x[r0:r1, grp * gpix : (grp + 1) * gpix]
                )
                xc = xap.tile([128, gpix], DT.bfloat16, tag="xc")
                nc.vector.tensor_copy(out=xc[0:nr, :], in_=xt[0:nr, :])
                for k in range(g):
                    pt = psT.tile([128, 128], DT.bfloat16, tag="pt")
                    nc.tensor.transpose(
                        out=pt[:, 0:nr],
                        in_=xc[0:nr, k * 128 : (k + 1) * 128],
                        identity=ident_sb[0:nr, 0:nr],
                    )
                    nc.scalar.copy(out=tA[:, k, r0:r1], in_=pt[:, 0:nr])
            for k in range(g):
                nc.vector.tensor_copy(out=tA[:, k, B:SORT_N], in_=sent_sb[:])

            # 36-substage all-ascending bitonic network, ping-pong tA <-> tB
            src, dst = tA, tB
            for s in range(8):
                L = 2 << s
                # mirror substage
                nb = SORT_N // L
                sv = src[:].rearrange("p g (nb two h) -> p g nb two h", two=2, h=L // 2)
                dv = dst[:].rearrange("p g (nb two h) -> p g nb two h", two=2, h=L // 2)
                lo_i, hi_i = sv[:, :, :, 0, :], sv[:, :, :, 1, :][:, :, :, ::-1]
                lo_o, hi_o = dv[:, :, :, 0, :], dv[:, :, :, 1, :][:, :, :, ::-1]
                nc.vector.tensor_tensor(out=lo_o, in0=lo_i, in1=hi_i, op=AF.min)
                nc.vector.tensor_tensor(out=hi_o, in0=lo_i, in1=hi_i, op=AF.max)
                src, dst = dst, src
                d = L // 4
                while d >= 1:
                    sv = src[:].rearrange("p g (nb two d) -> p g nb two d", two=2, d=d)
                    dv = dst[:].rearrange("p g (nb two d) -> p g nb two d", two=2, d=d)
                    lo_i, hi_i = sv[:, :, :, 0, :], sv[:, :, :, 1, :]
                    lo_o, hi_o = dv[:, :, :, 0, :], dv[:, :, :, 1, :]
                    nc.vector.tensor_tensor(out=lo_o, in0=lo_i, in1=hi_i, op=AF.min)
                    nc.vector.tensor_tensor(out=hi_o, in0=lo_i, in1=hi_i, op=AF.max)
                    src, dst = dst, src
                    d //= 2
            assert src is tA  # 36 substages: even number of swaps

            # run-length scoring on sorted rows
            eqt = rlp.tile([128, g, SORT_N], DT.bfloat16, tag="eq")
            nc.vector.memset(eqt[:, :, 0:1], 0.0)
            nc.vector.tensor_tensor(
                out=eqt[:, :, 1:SORT_N],
                in0=tA[:, :, 1:SORT_N],
                in1=tA[:, :, 0 : SORT_N - 1],
                op=AF.is_equal,
            )
            runpos = rlp.tile([128, g * SORT_N], DT.float32, tag="rp")
            eq_flat = eqt[:].rearrange("p g n -> p (g n)")
            # state = eq*state + eq  -> position within the current equal-run
            nc.vector.tensor_tensor_scan(
                out=runpos[:], data0=eq_flat, data1=eq_flat,
                initial=0.0, op0=AF.mult, op1=AF.add,
            )
            # score = runpos*256 - value; row max = (count-1, smallest value)
            nc.vector.scalar_tensor_tensor(
                out=runpos[:], in0=runpos[:], scalar=256.0,
                in1=tA[:].rearrange("p g n -> p (g n)"),
                op0=AF.mult, op1=AF.subtract,
            )
            nc.vector.tensor_reduce(
                out=score_cols[:, grp * g : (grp + 1) * g],
                in_=runpos[:].rearrange("p (g n) -> p g n", n=SORT_N),
                axis=mybir.AxisListType.X, op=AF.max,
            )

        # mode = 255 - ((smax + 255) mod 256), with mod done in exact fp32:
        # k = floor(y/256) via the round-to-int +2^23 trick (y' strictly
        # inside (k-0.5, k+0.5) so round(y') == k), then rem = y - 256*k.
        smax2 = consts.tile([128, nseg], DT.float32)
        nc.vector.tensor_scalar(
            out=smax2[:], in0=score_cols[:], scalar1=255.0, scalar2=0.0,
            op0=AF.add, op1=AF.add,
        )
        kf = consts.tile([128, nseg], DT.float32)
        nc.vector.tensor_scalar(
            out=kf[:], in0=smax2[:], scalar1=1.0 / 256.0,
            scalar2=-0.5 + 1.0 / 1024.0, op0=AF.mult, op1=AF.add,
        )
        nc.vector.tensor_scalar(
            out=kf[:], in0=kf[:], scalar1=float(2**23), scalar2=float(2**23),
            op0=AF.add, op1=AF.subtract,
        )
        modef = consts.tile([128, nseg], DT.float32)
        # rem = smax2 - 256*k ; then mode = 255 - rem
        nc.vector.scalar_tensor_tensor(
            out=modef[:], in0=kf[:], scalar=-256.0, in1=smax2[:],
            op0=AF.mult, op1=AF.add,
        )
        mode_bf = consts.tile([128, nseg], DT.bfloat16)
        nc.vector.tensor_scalar(
            out=mode_bf[:], in0=modef[:], scalar1=-1.0, scalar2=255.0,
            op0=AF.mult, op1=AF.add,
        )
        # transpose mode columns (partition t = segment t) and write mode_d
        # in pixel order: mode_d[t*128 + j] = mode of pixel t*128+j
        with (
            tc.tile_pool(name="psA", bufs=2, space="PSUM") as psA,
            tc.tile_pool(name="mrow", bufs=2) as mrow,
        ):
            for c in range((nseg + 127) // 128):
                w = min(128, nseg - c * 128)
                pt = psA.tile([128, 128], DT.bfloat16, tag="pt")
                nc.tensor.transpose(
                    out=pt[0:w, :], in_=mode_bf[:, c * 128 : c * 128 + w],
                    identity=ident_sb[:],
                )
                mr = mrow.tile([128, 128], DT.float32, tag="mr")
                nc.scalar.copy(out=mr[0:w, :], in_=pt[0:w, :])
                nc.sync.dma_start(
                    out=mode_d[c * 16384 : c * 16384 + w * 128].rearrange(
                        "(w f) -> w f", f=128
                    ),
                    in_=mr[0:w, :],
                )

        # ---- Phase B ----
        bgpool = ctx.enter_context(tc.tile_pool(name="bgp", bufs=2))
        xpool = ctx.enter_context(tc.tile_pool(name="xp", bufs=3))
        fgpool = ctx.enter_context(tc.tile_pool(name="fgp", bufs=3))

        row_splits = [(0, 128), (128, B)]
        for c in range(nchunk):
            col0 = c * chunk_pix
            bgs = bgpool.tile([128, chunk_pix], DT.float32, tag="bgs")
            nc.sync.dma_start(
                out=bgs[:],
                in_=mode_d[col0 : col0 + chunk_pix]
                .unsqueeze(0)
                .partition_broadcast(128),
            )
            nc.sync.dma_start(
                out=bg[0:128, col0 : col0 + chunk_pix], in_=bgs[:]
            )
            nc.sync.dma_start(
                out=bg[128:B, col0 : col0 + chunk_pix], in_=bgs[0 : B - 128, :]
            )
            for (r0, r1) in row_splits:
                nr = r1 - r0
                xt = xpool.tile([128, chunk_pix], DT.float32, tag="xt")
                nc.sync.dma_start(
                    out=xt[0:nr, :], in_=x[r0:r1, col0 : col0 + chunk_pix]
                )
                ft = fgpool.tile([128, chunk_pix], DT.float32, tag="ft")
                nc.vector.tensor_tensor(
                    out=ft[0:nr, :], in0=xt[0:nr, :], in1=bgs[0:nr, :],
                    op=AF.subtract,
                )
                nc.scalar.activation(
                    out=ft[0:nr, :], in_=ft[0:nr, :],
                    func=mybir.ActivationFunctionType.Abs,
                )
                nc.sync.dma_start(
                    out=fg[r0:r1, col0 : col0 + chunk_pix], in_=ft[0:nr, :]
                )
    nc.finalize()
    return nc


_NC_CACHE = {}


def _get_nc(p_core=P_CORE):
    if p_core not in _NC_CACHE:
        _NC_CACHE[p_core] = build_program(p_core)
    return _NC_CACHE[p_core]


def kernel(input):
    x = np.asarray(input)
    Bs, C, H, W = x.shape
    P = C * H * W
    xf = np.ascontiguousarray(x.reshape(Bs, P).astype(np.float32))
    pc = P // N_CORES
    in_maps = [
        {"x": np.ascontiguousarray(xf[:, i * pc : (i + 1) * pc])}
        for i in range(N_CORES)
    ]
    nc = _get_nc(pc)
    res = run_bass_kernel_spmd(nc, in_maps, list(range(N_CORES))).results
    bg = np.concatenate([res[i]["bg"] for i in range(N_CORES)], axis=1)
    fg = np.concatenate([res[i]["fg"] for i in range(N_CORES)], axis=1)
    bg = bg.reshape(Bs, C, H, W).astype(x.dtype, copy=False)
    fg = fg.reshape(Bs, C, H, W).astype(x.dtype, copy=False)
    return bg, fg


if __name__ == "__main__":
    rng = np.random.default_rng(0)
    xs = rng.integers(0, 256, size=(B, 3, 240, 320)).astype(np.float32)
    bg_, fg_ = kernel(xs)
    print(bg_.shape, fg_.shape)
